# revision 24
# baseline (speedup 1.0000x reference)
"""CIF (continuous integrate-and-fire) kernel for Trainium2, 8 NeuronCores.

Math: alphas = sigmoid(conv1d(x) @ lin_w + lin_b) folded to a 5-tap FIR with
w_eff[d,k] = sum_o conv_w[o,d,k]*lin_w[o].  The CIF scan output is expressed
in closed form: with S_t = cumsum(alpha), K_t = floor(S_t), frac_t = S_t-K_t,
G_t = cumsum(alpha_t * h_t), A_t = G_t - frac_t*h_t, the emitted segment j is
out[j] = A_{t_j} - A_{t_{j-1}} where t_j is the j-th fire step (K_t increments).
S is computed as 0.5*(t+1) + C_t with C_t = cumsum(alpha-0.5) so K/frac are
recovered at small-magnitude fp32 precision.  A rows are scattered to a dense
buffer P[K_t] via indirect DMA; out rows are adjacent differences of P.
Sharding: batch dim (16) split 2 rows per core across 8 cores.
"""

import numpy as np

B, T, D, P, NT = 16, 4096, 256, 128, 32
RPC = 2          # batch rows per core
NCORES = 8
KTAPS = 5
DUMP = T + 2     # P buffer: row 0 = zero, rows 1..T = data, row T+2 = dump

_built = None
_sync_patched = False


def _legalize_sync_json(bir):
    """This walrus build encodes at most ONE sync wait (and update) per
    non-DMA instruction.  Tile emits multi-wait instructions; hoist the
    excess waits onto single-wait NoOp instructions inserted just before
    (same engine queue => identical blocking semantics), and excess updates
    onto NoOps just after."""
    cnt = 0
    for fn in bir.get("functions", []):
        for blk in fn.get("blocks", []):
            new = []
            for inst in blk["instructions"]:
                si = inst.get("sync_info")
                eng = inst.get("engine")
                op = inst.get("opcode", "")
                exempt = not eng or not si
                pre, post = [], []
                if not exempt:
                    waits = si.get("on_wait") or []
                    if len(waits) > 1:
                        for w in waits[:-1]:
                            cnt += 1
                            pre.append({
                                "engine": eng, "opcode": "NoOp",
                                "name": f"I-syncw{cnt}", "ins": [], "outs": [],
                                "debug": inst.get("debug", 0),
                                "sync_info": {"on_wait": [w], "on_update": []},
                            })
                        si["on_wait"] = [waits[-1]]
                    ups = si.get("on_update") or []
                    if len(ups) > 1:
                        for u in ups[1:]:
                            cnt += 1
                            post.append({
                                "engine": eng, "opcode": "NoOp",
                                "name": f"I-syncu{cnt}", "ins": [], "outs": [],
                                "debug": inst.get("debug", 0),
                                "sync_info": {"on_wait": [], "on_update": [u]},
                            })
                        si["on_update"] = [ups[0]]
                new.extend(pre)
                new.append(inst)
                new.extend(post)
            blk["instructions"] = new
    return bir


def _install_sync_legalizer():
    global _sync_patched
    if _sync_patched:
        return
    import json as _json
    import concourse.bass_utils as _bu
    orig = _bu.compile_bir_kernel

    def wrapper(bir_json, tmpdir, neff_name="file.neff"):
        try:
            bir = _json.loads(
                bir_json.decode() if isinstance(bir_json, (bytes, bytearray))
                else bir_json
            )
            _legalize_sync_json(bir)
            bir_json = _json.dumps(bir).encode()
        except Exception:
            pass
        return orig(bir_json, tmpdir, neff_name=neff_name)

    _bu.compile_bir_kernel = wrapper
    try:
        import concourse.bass2jax as _b2j
        _b2j.compile_bir_kernel = wrapper
    except Exception:
        pass
    _sync_patched = True


def _build_kernel():
    import concourse.bass as bass
    import concourse.mybir as mybir
    from concourse import masks
    from concourse.tile import TileContext

    f32 = mybir.dt.float32
    i32 = mybir.dt.int32
    AF = mybir.ActivationFunctionType
    Op = mybir.AluOpType

    nc = bass.Bass()
    x_in = nc.dram_tensor("x", [RPC, T, D], f32, kind="ExternalInput")
    weff_in = nc.dram_tensor("weff", [D, KTAPS], f32, kind="ExternalInput")
    linb_in = nc.dram_tensor("linb", [1], f32, kind="ExternalInput")
    out_t = nc.dram_tensor("out", [RPC, T, D], f32, kind="ExternalOutput")
    len_t = nc.dram_tensor("len", [RPC, 1], i32, kind="ExternalOutput")
    pbufs = [nc.dram_tensor(f"pbuf{r}", [T + 3, D], f32) for r in range(RPC)]

    with TileContext(nc) as tc:
        with (
            tc.tile_pool(name="const", bufs=1) as cp,
            tc.tile_pool(name="persist", bufs=1) as pp,
            tc.tile_pool(name="work", bufs=3) as wp,
            tc.tile_pool(name="ps_xtl", bufs=2, space="PSUM") as ps_xtl,
            tc.tile_pool(name="ps_g", bufs=2, space="PSUM") as ps_g,
            tc.tile_pool(name="ps_small", bufs=2, space="PSUM") as ps_small,
        ):
            # ---- constants ----
            ident = cp.tile([P, P], f32)
            masks.make_identity(nc, ident[:])
            utri = cp.tile([P, P], f32)
            masks.make_upper_triangular(nc, utri[:], val=1.0, diag=True)
            ones_row = cp.tile([1, P], f32)
            nc.vector.memset(ones_row[:], 1.0)
            ones_col = cp.tile([P, 1], f32)
            nc.vector.memset(ones_col[:], 1.0)
            onesq = cp.tile([P, P], f32)
            nc.vector.memset(onesq[:], 1.0)
            zeros_row_nt = cp.tile([1, NT], f32)
            nc.vector.memset(zeros_row_nt[:], 0.0)
            zero_tile_d = cp.tile([P, D], f32)
            nc.vector.memset(zero_tile_d[:], 0.0)
            dump_tile = cp.tile([P, NT], f32)
            nc.vector.memset(dump_tile[:], float(DUMP))

            # shift matrices: M[t, t'] = 1 iff t = t' + d  (affine keep iff
            # t - t' + base == 0 with base = -d); wrap variants cross tiles.
            def shift_mat(base, tag):
                m = cp.tile([P, P], f32, tag=tag)
                nc.gpsimd.affine_select(
                    out=m[:], in_=onesq[:],
                    compare_op=Op.is_equal, fill=0.0,
                    base=base, pattern=[[-1, P]], channel_multiplier=1,
                )
                return m

            m_main = {}   # delta -> matrix (within-tile part)
            m_wrap = {}   # delta -> matrix (adjacent-tile part)
            for dlt in (-2, -1, 1, 2):
                m_main[dlt] = shift_mat(-dlt, f"mmain{dlt}")
                if dlt > 0:
                    m_wrap[dlt] = shift_mat(P - dlt, f"mwrap{dlt}")
                else:
                    m_wrap[dlt] = shift_mat(-(dlt + P), f"mwrap{dlt}")

            w0 = cp.tile([P, KTAPS], f32)
            w1 = cp.tile([P, KTAPS], f32)
            nc.sync.dma_start(out=w0[:], in_=weff_in[0:P, :])
            nc.sync.dma_start(out=w1[:], in_=weff_in[P:D, :])
            linb_b = cp.tile([P, 1], f32)
            nc.sync.dma_start(
                out=linb_b[:], in_=linb_in[None, :].to_broadcast([P, 1])
            )

            # per-partition consts: halfflag = 0.5*((p+1)%2),
            # dint[p,i] = 64*i + 0.5*((p+1)-(p+1)%2)
            # generic floor(x) for x > 0 that works under any f32->int cast
            # rounding mode: fl0 = cast(x); fix = (x - fl0 < 0); fl = fl0 - fix
            def emit_floor(pool, src_ap, shape, tag):
                fi = pool.tile(shape, i32, tag=tag + "_i")
                nc.vector.tensor_copy(fi[:], src_ap)
                ff = pool.tile(shape, f32, tag=tag + "_f")
                nc.vector.tensor_copy(ff[:], fi[:])
                dd = pool.tile(shape, f32, tag=tag + "_d")
                nc.vector.tensor_tensor(out=dd[:], in0=src_ap, in1=ff[:],
                                        op=Op.subtract)
                fx = pool.tile(shape, f32, tag=tag + "_x")
                nc.vector.tensor_scalar(out=fx[:], in0=dd[:], scalar1=0.0,
                                        scalar2=None, op0=Op.is_lt)
                nc.vector.tensor_tensor(out=ff[:], in0=ff[:], in1=fx[:],
                                        op=Op.subtract)
                return ff  # floor(src)

            pi_i = cp.tile([P, 1], i32)
            nc.gpsimd.iota(pi_i[:], pattern=[[0, 1]], base=1, channel_multiplier=1)
            p1f = cp.tile([P, 1], f32)
            nc.vector.tensor_copy(p1f[:], pi_i[:])
            p1h = cp.tile([P, 1], f32)
            nc.vector.tensor_scalar_mul(p1h[:], p1f[:], 0.5)
            fh = emit_floor(cp, p1h[:], [P, 1], "fhfl")   # floor((p+1)/2)
            halfflag = cp.tile([P, 1], f32)
            nc.vector.tensor_tensor(
                out=halfflag[:], in0=p1h[:], in1=fh[:], op=Op.subtract
            )
            i64 = cp.tile([P, NT], i32)
            nc.gpsimd.iota(i64[:], pattern=[[64, NT]], base=0, channel_multiplier=0)
            # dint includes a -64 correction for the +64 shift applied to C
            # before the mod (keeps the mod argument positive on any HW
            # fmod-sign convention).
            dint = cp.tile([P, NT], f32)
            nc.vector.tensor_copy(dint[:], i64[:])
            nc.vector.tensor_scalar_add(dint[:], dint[:], fh[:, 0:1])
            nc.vector.tensor_scalar_sub(dint[:], dint[:], 64.0)

            # warmup matmul touching the gpsimd-built consts so PE observes
            # their semaphore once; real matmuls then carry fewer waits
            # (LDWEIGHTS fits only one sync wait).
            ps_warm = ps_small.tile([1, P], f32, tag="ptot")
            nc.tensor.matmul(ps_warm[0:1, 0:P], lhsT=m_main[-1][:, 0:1],
                             rhs=m_main[1][:], start=True, stop=True)
            nc.tensor.matmul(ps_warm[0:1, 0:P], lhsT=m_main[-2][:, 0:1],
                             rhs=m_main[2][:], start=True, stop=True)
            nc.tensor.matmul(ps_warm[0:1, 0:P], lhsT=m_wrap[-1][:, 0:1],
                             rhs=m_wrap[1][:], start=True, stop=True)
            nc.tensor.matmul(ps_warm[0:1, 0:P], lhsT=m_wrap[-2][:, 0:1],
                             rhs=m_wrap[2][:], start=True, stop=True)
            nc.tensor.matmul(ps_warm[0:1, 0:P], lhsT=ident[:, 0:1],
                             rhs=utri[:], start=True, stop=True)
            nc.tensor.matmul(ps_warm[0:1, 0:1], lhsT=ones_col[:],
                             rhs=dint[:, 0:1], start=True, stop=True)
            # observe the w0/w1 DMA queues too (one queue per matmul)
            nc.tensor.matmul(ps_warm[0:1, 0:KTAPS], lhsT=w0[:, 0:1],
                             rhs=w0[:], start=True, stop=True)
            nc.tensor.matmul(ps_warm[0:1, 0:KTAPS], lhsT=w1[:, 0:1],
                             rhs=w1[:], start=True, stop=True)

            for r in range(RPC):
                xr = x_in[r]  # [T, D]
                # ---------- phase 1: per-tap logits per tile ----------
                xtiles = []
                lall = pp.tile([P, KTAPS * NT], f32, tag=f"lall{r}")
                for i in range(NT):
                    xt = pp.tile([P, D], f32, tag=f"x{r}_{i}")
                    xtiles.append(xt)
                    nc.sync.dma_start(out=xt[:], in_=xr[i * P:(i + 1) * P, :])
                    pxt = ps_xtl.tile([P, D + KTAPS], f32, tag="xtl")
                    nc.tensor.transpose(pxt[:, 0:P], xt[:, 0:P], ident[:])
                    nc.tensor.transpose(pxt[:, P:D], xt[:, P:D], ident[:])
                    xts = wp.tile([P, D], f32, tag="xts")
                    nc.vector.tensor_copy(xts[:], pxt[:, 0:D])
                    nc.tensor.matmul(
                        pxt[:, D:D + KTAPS], lhsT=xts[:, 0:P], rhs=w0[:],
                        start=True, stop=False,
                    )
                    nc.tensor.matmul(
                        pxt[:, D:D + KTAPS], lhsT=xts[:, P:D], rhs=w1[:],
                        start=False, stop=True,
                    )
                    nc.vector.tensor_copy(
                        lall[:, i * KTAPS:(i + 1) * KTAPS], pxt[:, D:D + KTAPS]
                    )

                # ---------- phase 2: tap combine (shift matmuls), sigmoid ----------
                plo = ps_small.tile([P, NT], f32, tag="small")
                # logit[t] = sum_k Ltap[t + (k-2), k]
                mm_list = [(ident, lall[:, 2::KTAPS], plo[:])]
                for k in range(KTAPS):
                    dlt = k - 2
                    if dlt == 0:
                        continue
                    mm_list.append((m_main[dlt], lall[:, k::KTAPS], plo[:]))
                    if dlt > 0:
                        # rows >= P-dlt of tile i come from tile i+1
                        mm_list.append((
                            m_wrap[dlt], lall[:, KTAPS + k::KTAPS],
                            plo[:, 0:NT - 1],
                        ))
                    else:
                        # rows < -dlt of tile i come from tile i-1
                        mm_list.append((
                            m_wrap[dlt],
                            lall[:, k:k + KTAPS * (NT - 1):KTAPS],
                            plo[:, 1:NT],
                        ))
                for idx, (lhs, rhs, outap) in enumerate(mm_list):
                    nc.tensor.matmul(
                        outap, lhsT=lhs[:], rhs=rhs,
                        start=(idx == 0), stop=(idx == len(mm_list) - 1),
                    )

                # stage logits to SBUF on DVE so ACT never reads PSUM (keeps
                # matmul WAR waits on a single engine)
                lo_sb = wp.tile([P, NT], f32, tag="lo_sb")
                nc.vector.tensor_copy(lo_sb[:], plo[:])
                alpha = pp.tile([P, NT], f32, tag=f"alpha{r}")
                nc.scalar.activation(
                    alpha[:], lo_sb[:], AF.Sigmoid, bias=linb_b[:, 0:1], scale=1.0
                )
                am5 = wp.tile([P, NT], f32, tag="am5")
                nc.vector.tensor_scalar_sub(am5[:], alpha[:], 0.5)

                # ---------- C-scan ----------
                psc = ps_small.tile([P, NT], f32, tag="small")
                nc.tensor.matmul(psc[:], lhsT=utri[:], rhs=am5[:],
                                 start=True, stop=True)
                # tile totals at partition 0 (partition-127 reads are illegal
                # for compute engines)
                ptc = ps_small.tile([1, NT], f32, tag="ptot")
                nc.tensor.matmul(ptc[:], lhsT=ones_col[:], rhs=am5[:],
                                 start=True, stop=True)
                offc = pp.tile([1, NT + 1], f32, tag=f"offc{r}")
                nc.vector.memset(offc[0:1, 0:1], 0.0)
                nc.vector.tensor_tensor_scan(
                    out=offc[0:1, 1:NT + 1],
                    data0=ptc[0:1, :],
                    data1=zeros_row_nt[:],
                    initial=0.0,
                    op0=Op.add,
                    op1=Op.add,
                )
                psb = ps_small.tile([P, NT], f32, tag="small")
                nc.tensor.matmul(psb[:], lhsT=ones_row[:], rhs=offc[0:1, 0:NT],
                                 start=True, stop=True)
                cc = pp.tile([P, NT], f32, tag=f"cc{r}")
                nc.vector.tensor_copy(cc[:], psc[:])
                nc.vector.tensor_tensor(out=cc[:], in0=cc[:], in1=psb[:],
                                        op=Op.add)
                # shift +64 so the mod argument is positive
                nc.vector.tensor_scalar_add(cc[:], cc[:], 64.0)

                # ---------- K/frac via floor (exact given C) ----------
                kc = emit_floor(wp, cc[:], [P, NT], "ccfl")  # floor(C+64)
                rr = wp.tile([P, NT], f32, tag="rr")
                nc.vector.tensor_tensor(out=rr[:], in0=cc[:], in1=kc[:],
                                        op=Op.subtract)
                uu = wp.tile([P, NT], f32, tag="uu")
                nc.vector.tensor_scalar_add(uu[:], rr[:], halfflag[:, 0:1])
                mm_ = wp.tile([P, NT], f32, tag="mm")
                nc.vector.tensor_scalar(
                    out=mm_[:], in0=uu[:], scalar1=1.0, scalar2=None, op0=Op.is_ge
                )
                frac = pp.tile([P, NT], f32, tag=f"frac{r}")
                nc.vector.tensor_tensor(out=frac[:], in0=uu[:], in1=mm_[:],
                                        op=Op.subtract)
                kk = pp.tile([P, NT], f32, tag=f"kk{r}")
                nc.vector.tensor_tensor(out=kk[:], in0=kc[:], in1=mm_[:], op=Op.add)
                nc.vector.tensor_tensor(out=kk[:], in0=kk[:], in1=dint[:], op=Op.add)

                # ---------- fire = K_t > K_{t-1} via shift matmul ----------
                pks = ps_small.tile([P, NT], f32, tag="small")
                nc.tensor.matmul(pks[:], lhsT=m_main[-1][:], rhs=kk[:],
                                 start=True, stop=False)
                nc.tensor.matmul(pks[:, 1:NT], lhsT=m_wrap[-1][:],
                                 rhs=kk[:, 0:NT - 1], start=False, stop=True)
                fire = pp.tile([P, NT], f32, tag=f"fire{r}")
                nc.vector.tensor_tensor(out=fire[:], in0=kk[:], in1=pks[:],
                                        op=Op.is_gt)

                # offs = DUMP + fire * (K - DUMP)
                offs_f = pp.tile([P, NT], f32, tag=f"offsf{r}")
                nc.vector.tensor_scalar_sub(offs_f[:], kk[:], float(DUMP))
                nc.vector.tensor_tensor(out=offs_f[:], in0=offs_f[:],
                                        in1=fire[:], op=Op.mult)
                nc.vector.tensor_scalar_add(offs_f[:], offs_f[:], float(DUMP))
                offs_i = pp.tile([P, NT], i32, tag=f"offsi{r}")
                nc.vector.tensor_copy(offs_i[:], offs_f[:])

                # len = max(K[last], 1)
                nstage = wp.tile([1, 1], f32, tag="nstage")
                nc.sync.dma_start(out=nstage[:],
                                  in_=kk[P - 1:P, NT - 1:NT])
                nf = wp.tile([1, 1], f32, tag="nf")
                nc.vector.tensor_scalar_max(nf[:], nstage[:], 1.0)
                ni = pp.tile([1, 1], i32, tag=f"ni{r}")
                nc.vector.tensor_copy(ni[:], nf[:])
                nc.sync.dma_start(out=len_t[r, 0:1], in_=ni[0:1, 0])

                # ---------- phase 5: G chain, A tiles, scatter ----------
                # zero-init P so unfired rows read as zero
                for j in range(NT):
                    nc.sync.dma_start(
                        out=pbufs[r][j * P:(j + 1) * P, :], in_=zero_tile_d[:]
                    )
                nc.sync.dma_start(out=pbufs[r][NT * P:T + 3, :],
                                  in_=zero_tile_d[0:3, :])
                prev_stage = None
                for i in range(NT):
                    ah = wp.tile([P, D], f32, tag="ah")
                    nc.vector.tensor_scalar_mul(
                        ah[:], xtiles[i][:], alpha[:, i:i + 1]
                    )
                    pg = ps_g.tile([P, D], f32, tag="g")
                    if prev_stage is None:
                        nc.tensor.matmul(pg[:], lhsT=utri[:], rhs=ah[:],
                                         start=True, stop=True)
                    else:
                        nc.tensor.matmul(pg[:], lhsT=utri[:], rhs=ah[:],
                                         start=True, stop=False)
                        nc.tensor.matmul(pg[:], lhsT=ones_row[:],
                                         rhs=prev_stage[:],
                                         start=False, stop=True)
                    # running row-total chain kept at partition 0: an M=1
                    # ones-column matmul gives this tile's total there.
                    ptg = ps_small.tile([1, D], f32, tag="ptot")
                    nc.tensor.matmul(ptg[:], lhsT=ones_col[:], rhs=ah[:],
                                     start=True, stop=True)
                    stage = wp.tile([1, D], f32, tag="stage")
                    if prev_stage is None:
                        nc.vector.tensor_copy(stage[:], ptg[:])
                    else:
                        nc.vector.tensor_tensor(out=stage[:],
                                                in0=prev_stage[:],
                                                in1=ptg[:], op=Op.add)
                    prev_stage = stage
                    tmp = wp.tile([P, D], f32, tag="tmp")
                    nc.vector.tensor_scalar_mul(
                        tmp[:], xtiles[i][:], frac[:, i:i + 1]
                    )
                    at = wp.tile([P, D], f32, tag="at")
                    nc.vector.tensor_tensor(out=at[:], in0=pg[:], in1=tmp[:],
                                            op=Op.subtract)
                    nc.gpsimd.indirect_dma_start(
                        out=pbufs[r][:, :],
                        out_offset=bass.IndirectOffsetOnAxis(
                            ap=offs_i[:, i:i + 1], axis=0
                        ),
                        in_=at[:],
                        in_offset=None,
                    )

                # ---------- phase 6: adjacent differences ----------
                for j in range(NT):
                    pt = wp.tile([P, D], f32, tag="pt")
                    nc.sync.dma_start(
                        out=pt[:], in_=pbufs[r][1 + j * P:1 + (j + 1) * P, :]
                    )
                    pm = wp.tile([P, D], f32, tag="pm")
                    nc.sync.dma_start(
                        out=pm[:], in_=pbufs[r][j * P:(j + 1) * P, :]
                    )
                    od = wp.tile([P, D], f32, tag="od")
                    nc.vector.tensor_tensor(out=od[:], in0=pt[:], in1=pm[:],
                                            op=Op.subtract)
                    nc.sync.dma_start(
                        out=out_t[r, j * P:(j + 1) * P, :], in_=od[:]
                    )
    return nc


def kernel(encoder_outputs, conv_w, conv_b, lin_w, lin_b):
    global _built
    _install_sync_legalizer()
    from concourse.bass_utils import run_bass_kernel_spmd

    x = np.ascontiguousarray(encoder_outputs, dtype=np.float32)
    w_eff = np.einsum(
        "oik,o->ik", conv_w.astype(np.float64), lin_w[:, 0].astype(np.float64)
    ).astype(np.float32)
    linb = np.ascontiguousarray(lin_b, dtype=np.float32)

    if _built is None:
        _built = _build_kernel()
    nc = _built

    in_maps = [
        {"x": x[c * RPC:(c + 1) * RPC], "weff": w_eff, "linb": linb}
        for c in range(NCORES)
    ]
    res = run_bass_kernel_spmd(nc, in_maps, list(range(NCORES)))
    out = np.zeros((B, T, D), np.float32)
    lens = np.zeros((B,), np.int32)
    for c in range(NCORES):
        o = res.results[c]["out"]
        l = res.results[c]["len"].reshape(-1)
        for r in range(RPC):
            b = c * RPC + r
            n = int(l[r])
            lens[b] = n
            out[b, :n] = o[r, :n]
    return out, lens
